# revision 1
# baseline (speedup 1.0000x reference)
"""Quantized-weight batched linear: out[b,n,m] = sum_k deq(qweight)[n,k] * x[b,k,m].

Strategy:
  - Host: dequantize weight (fp32, exact same formula as the oracle), transpose
    to (K, N), round weights + activations to bf16.
  - Device (8 cores, data-parallel over batch B=64 -> 8 batches/core):
    PE bf16 matmuls, K accumulated in PSUM over 8 chunks of 128,
    N tiled 8x128 (PSUM partitions), M tiled 2x512 (PSUM bank free-dim).
  - Gather core outputs along batch -> (64, 1024, 1024) fp32.
"""

import numpy as np
import ml_dtypes

N = 1024  # output rows (weight rows)
K = 1024  # reduction dim
M = 1024  # columns of x per batch
NGROUP = 16
GS = K // NGROUP
B = 64
NCORES = 8
BPC = B // NCORES  # batches per core

_CACHE = {}
LAST_RESULT = None  # BassKernelResults of the most recent run (for profiling)


def _build_nc(bpc=BPC, k=K, n=N, m=M):
    import concourse.mybir as mybir
    import concourse.tile as tile
    from concourse import bacc

    kc = k // 128   # contraction chunks (partition dim)
    nt = n // 128   # output-row tiles (PSUM partition dim)
    mt = m // 512   # moving free-dim tiles (one PSUM bank each)

    nc = bacc.Bacc(None, target_bir_lowering=False, debug=False)
    wt = nc.dram_tensor("wt", [k, n], mybir.dt.bfloat16, kind="ExternalInput")
    xs = nc.dram_tensor("xs", [bpc, k, m], mybir.dt.bfloat16, kind="ExternalInput")
    out = nc.dram_tensor("out", [bpc, n, m], mybir.dt.float32, kind="ExternalOutput")

    nh = 2           # process each batch in nh groups of n0 tiles
    npg = nt // nh   # n0 tiles per group; npg*mt PSUM banks live at once

    with tile.TileContext(nc) as tc:
        with (
            tc.tile_pool(name="wpool", bufs=1) as wpool,
            tc.tile_pool(name="xpool", bufs=2 * kc) as xpool,
            tc.tile_pool(name="opool", bufs=8) as opool,
            tc.tile_pool(name="psum", bufs=8, space="PSUM") as psum_pool,
        ):
            # All loads on the sync HWDGE queue, all stores on the scalar
            # HWDGE queue (static DMAs occupy the issuing sequencer for the
            # transfer; separate streams avoid head-of-line blocking and
            # Tile's cross-queue ordering waits).
            #
            # Startup: interleave weight slab k with x[batch0] chunk k so
            # chunk k's matmuls (k-outer order below gives 1.7us of PE work
            # per chunk) never wait on later transfers.
            wsb = []
            xcur = []
            for kk in range(kc):
                wtile = wpool.tile([128, n], mybir.dt.bfloat16, tag=f"w{kk}", name=f"w{kk}")
                nc.sync.dma_start(out=wtile[:], in_=wt[kk * 128:(kk + 1) * 128, :])
                wsb.append(wtile)
                xt = xpool.tile([128, m], mybir.dt.bfloat16, tag="x", name=f"x0_{kk}")
                nc.sync.dma_start(out=xt[:], in_=xs[0, kk * 128:(kk + 1) * 128, :])
                xcur.append(xt)

            for b in range(bpc):
                if b + 1 < bpc:
                    xnext = []
                    for kk in range(kc):
                        xt = xpool.tile([128, m], mybir.dt.bfloat16, tag="x", name=f"x{b + 1}_{kk}")
                        nc.sync.dma_start(out=xt[:], in_=xs[b + 1, kk * 128:(kk + 1) * 128, :])
                        xnext.append(xt)
                else:
                    xnext = None

                # Last batch tapers group size so the final PSUM drain (which
                # nothing overlaps) is only one n0 tile instead of four.
                groups = [4, 2, 1, 1] if b == bpc - 1 else [npg] * nh
                final_group = None if b != bpc - 1 else len(groups) - 1
                n0_base = 0
                for h, gsz in enumerate(groups):
                    # k-outer accumulation into gsz*mt PSUM banks: every x
                    # chunk is fully consumed (gsz*mt matmuls) on arrival.
                    ps = {}
                    for j in range(gsz):
                        for m0 in range(mt):
                            ps[j, m0] = psum_pool.tile(
                                [128, 512], mybir.dt.float32, tag="ps", name=f"ps{b}_{h}_{j}_{m0}"
                            )
                    for kk in range(kc):
                        for j in range(gsz):
                            n0 = n0_base + j
                            lhsT = wsb[kk][:, n0 * 128:(n0 + 1) * 128]
                            for m0 in range(mt):
                                nc.tensor.matmul(
                                    ps[j, m0][:],
                                    lhsT,
                                    xcur[kk][:, m0 * 512:(m0 + 1) * 512],
                                    start=(kk == 0),
                                    stop=(kk == kc - 1),
                                )
                    for j in range(gsz):
                        n0 = n0_base + j
                        for m0 in range(mt):
                            ot = opool.tile([128, 512], mybir.dt.float32, tag="o", name=f"o{b}_{n0}_{m0}")
                            if h == final_group:
                                # Parallel drain of the very last tiles.
                                cp = (nc.vector.tensor_copy if m0 % 2 == 0
                                      else nc.scalar.copy)
                                st_eng = nc.sync
                            else:
                                cp = nc.vector.tensor_copy
                                st_eng = nc.scalar
                            cp(ot[:], ps[j, m0][:])
                            st_eng.dma_start(
                                out=out[b, n0 * 128:(n0 + 1) * 128, m0 * 512:(m0 + 1) * 512],
                                in_=ot[:],
                            )
                    n0_base += gsz
                xcur = xnext
    nc.compile()
    return nc


def _dequant_wt(qweight, qrange, qmin):
    # Matches reference: w = q * qrange + qmin per (row, group), fp32.
    q = np.asarray(qweight).astype(np.float32).reshape(N, NGROUP, GS)
    qr = np.asarray(qrange).astype(np.float32).reshape(N, NGROUP, 1)
    qm = np.asarray(qmin).astype(np.float32).reshape(N, NGROUP, 1)
    w = (q * qr + qm).reshape(N, K)
    return np.ascontiguousarray(w.T).astype(ml_dtypes.bfloat16)  # (K, N)


def _ensure_axon_hooks():
    """run_bass_kernel_spmd(trace=True) imports antenv.axon_hooks, which some
    images lack; provide a stub (and register the real NTFF hook if the boot
    package is present) so tracing degrades gracefully instead of crashing."""
    try:
        import antenv.axon_hooks  # noqa: F401
        return
    except ImportError:
        pass
    try:
        import sys
        import types

        import antenv

        mod = types.ModuleType("antenv.axon_hooks")
        mod._hook = None
        mod.set_axon_ntff_profile_hook = lambda h: setattr(mod, "_hook", h)
        mod.get_axon_ntff_profile_hook = lambda: mod._hook
        sys.modules["antenv.axon_hooks"] = mod
        antenv.axon_hooks = mod
        try:
            from trn_agent_boot.trn_boot import _ntff_profile_via_ctypes

            mod._hook = _ntff_profile_via_ctypes("/opt/axon/libaxon_pjrt.so")
        except Exception:
            pass
    except Exception:
        pass


def kernel(x, qweight, qrange, qmin):
    global LAST_RESULT
    _ensure_axon_hooks()
    from concourse.bass_utils import run_bass_kernel_spmd

    wt_host = _dequant_wt(qweight, qrange, qmin)
    xb = np.asarray(x).astype(ml_dtypes.bfloat16)  # (B, K, M)

    if "nc" not in _CACHE:
        _CACHE["nc"] = _build_nc()
    nc = _CACHE["nc"]

    in_maps = [
        {"wt": wt_host, "xs": np.ascontiguousarray(xb[c * BPC:(c + 1) * BPC])}
        for c in range(NCORES)
    ]
    LAST_RESULT = run_bass_kernel_spmd(nc, in_maps, core_ids=list(range(NCORES)))
    outs = [r["out"] for r in LAST_RESULT.results]
    return np.ascontiguousarray(np.concatenate(outs, axis=0)).astype(np.float32, copy=False)

